# revision 4
# baseline (speedup 1.0000x reference)
"""MoE layer (top-2 of 8 experts + shared expert) as a Bass/Tile kernel on 8 TRN2 cores.

Strategy (expert parallelism, per the sharding hint):
  - Host computes the tiny gating network (softmax -> top-2 -> renormalize) and
    builds the all-to-all token dispatch: core e receives the tokens routed to
    expert e (padded to a fixed capacity C), pre-transposed to [d_model, C].
  - Core e runs expert e's FFN on its tokens (silu(x@W1 * x@W3) @ W2), scales
    each output token by its gate weight, and also runs a 512-token slice of the
    shared expert (token-parallel across the 8 cores).
  - Host scatter-adds the two expert contributions per token and the shared
    output back into the full [T, d] result.

All heavy FLOPs (expert FFNs + shared FFN) run on device; the host only does
O(T*E) gating math and O(T*d) data movement.
"""

import os
import sys

for _p in ("/opt/trn_rl_repo",):
    if _p not in sys.path and os.path.isdir(_p):
        sys.path.insert(0, _p)

import numpy as np
import ml_dtypes

import concourse.bass as bass
import concourse.mybir as mybir
import concourse.tile as tile
from concourse import bacc
from concourse.bass_utils import run_bass_kernel_spmd

P = 128
D = 1024          # d_model
F = 2048          # d_ff per expert
FS = 4096         # shared expert hidden
E = 8             # experts == cores
TOPK = 2
NCH = 512         # free-dim chunk (one PSUM bank of fp32)
TS = 512          # shared-expert tokens per core (T / 8)
T = 4096

f32 = mybir.dt.float32
bf16 = mybir.dt.bfloat16

# matmul dtype config: "bf16" or "f32r" (fp32 storage, reduced-precision matmul)
MM_CFG = os.environ.get("MOE_MM_CFG", "bf16")

_COMPILED: dict = {}


def _np_mm_dtype(cfg):
    return ml_dtypes.bfloat16 if cfg == "bf16" else np.float32


def _bir_mm_dtype(cfg):
    return bf16 if cfg == "bf16" else mybir.dt.float32r


def build_program(C: int, cfg: str):
    """Build the per-core Bass program for capacity C."""
    assert C % NCH == 0
    NC = C // NCH           # token chunks in expert path
    mdt = _bir_mm_dtype(cfg)
    # for f32r the SBUF tiles are float32 and get bitcast at the matmul
    sdt = bf16 if cfg == "bf16" else f32

    nc = bacc.Bacc("TRN2", target_bir_lowering=False, debug=False, num_devices=E)

    # ---- per-core inputs ----
    xgT = nc.dram_tensor("xgT", [D, C], sdt, kind="ExternalInput")       # gathered expert tokens (transposed)
    gw = nc.dram_tensor("gw", [1, C], f32, kind="ExternalInput")         # gate weight per slot
    w1 = nc.dram_tensor("w1", [D, F], sdt, kind="ExternalInput")
    w3 = nc.dram_tensor("w3", [D, F], sdt, kind="ExternalInput")
    w2 = nc.dram_tensor("w2", [F, D], sdt, kind="ExternalInput")
    b1 = nc.dram_tensor("b1", [F], f32, kind="ExternalInput")
    b3 = nc.dram_tensor("b3", [F], f32, kind="ExternalInput")
    b2 = nc.dram_tensor("b2", [D], f32, kind="ExternalInput")
    xsT = nc.dram_tensor("xsT", [D, TS], sdt, kind="ExternalInput")      # shared-expert token slice (transposed)
    ws1 = nc.dram_tensor("ws1", [D, FS], sdt, kind="ExternalInput")
    ws3 = nc.dram_tensor("ws3", [D, FS], sdt, kind="ExternalInput")
    ws2 = nc.dram_tensor("ws2", [FS, D], sdt, kind="ExternalInput")
    bs1 = nc.dram_tensor("bs1", [FS], f32, kind="ExternalInput")
    bs3 = nc.dram_tensor("bs3", [FS], f32, kind="ExternalInput")
    bs2 = nc.dram_tensor("bs2", [D], f32, kind="ExternalInput")

    # ---- per-core outputs ----
    yT = nc.dram_tensor("yT", [D, C], f32, kind="ExternalOutput")        # gate-scaled expert output
    ysT = nc.dram_tensor("ysT", [D, TS], f32, kind="ExternalOutput")     # shared-expert output slice

    DT = D // P    # 8  d-model tiles
    FT = F // P    # 16 expert-hidden tiles
    FST = FS // P  # 32 shared-hidden tiles

    def mmv(ap):
        # view an SBUF tile as the matmul dtype
        return ap.bitcast(mybir.dt.float32r) if cfg == "f32r" else ap

    with tile.TileContext(nc) as tc:
        with (
            tc.tile_pool(name="consts", bufs=1) as consts,
            tc.tile_pool(name="xg", bufs=1) as xgp,
            tc.tile_pool(name="wres", bufs=1) as wres,
            tc.tile_pool(name="wstream", bufs=2) as wstream,
            tc.tile_pool(name="abuf", bufs=1) as abufp,
            tc.tile_pool(name="htmp", bufs=3) as htmp,
            tc.tile_pool(name="ytmp", bufs=3) as ytmp,
            tc.tile_pool(name="ph", bufs=2, space="PSUM") as php,
            tc.tile_pool(name="py", bufs=2, space="PSUM") as pyp,
        ):
            # ---------- constants ----------
            gw_sb = consts.tile([P, C], f32)
            nc.sync.dma_start(gw_sb[:], gw[:].to_broadcast([P, C]))
            b1_sb = consts.tile([P, FT], f32)
            nc.sync.dma_start(b1_sb[:], b1[:].rearrange("(o p) -> p o", p=P))
            b3_sb = consts.tile([P, FT], f32)
            nc.sync.dma_start(b3_sb[:], b3[:].rearrange("(o p) -> p o", p=P))
            b2_sb = consts.tile([P, DT], f32)
            nc.sync.dma_start(b2_sb[:], b2[:].rearrange("(o p) -> p o", p=P))
            bs1_sb = consts.tile([P, FST], f32)
            nc.sync.dma_start(bs1_sb[:], bs1[:].rearrange("(o p) -> p o", p=P))
            bs3_sb = consts.tile([P, FST], f32)
            nc.sync.dma_start(bs3_sb[:], bs3[:].rearrange("(o p) -> p o", p=P))
            bs2_sb = consts.tile([P, DT], f32)
            nc.sync.dma_start(bs2_sb[:], bs2[:].rearrange("(o p) -> p o", p=P))

            # ---------- expert-path inputs ----------
            xg_sb = xgp.tile([P, DT, C], sdt)
            nc.sync.dma_start(xg_sb[:], xgT[:].rearrange("(o p) c -> p o c", p=P))

            # resident expert weights, one clean DMA each (rows contiguous)
            w1_sb = wres.tile([P, DT, F], sdt, tag="w1res")
            nc.sync.dma_start(w1_sb[:], w1[:].rearrange("(o p) f -> p o f", p=P))
            w3_sb = wres.tile([P, DT, F], sdt, tag="w3res")
            nc.sync.dma_start(w3_sb[:], w3[:].rearrange("(o p) f -> p o f", p=P))
            w2_sb = wres.tile([P, FT, D], sdt, tag="w2res")
            nc.sync.dma_start(w2_sb[:], w2[:].rearrange("(o p) d -> p o d", p=P))

            # a-buffer: reused as [P, FT, NCH] per expert chunk (2 slots) and
            # as [P, FST, NCH] for the shared expert afterwards
            ab = abufp.tile([P, FST * NCH], sdt, tag="abuf")

            def a_expert(n):
                off = (n % 2) * (FT * NCH)
                return ab[:, off : off + FT * NCH].rearrange(
                    "p (f n) -> p f n", f=FT
                )

            a_shared = ab[:, : FST * NCH].rearrange("p (f n) -> p f n", f=FST)

            # ---------- expert path ----------
            for n in range(NC):
                a_n = a_expert(n)
                ncols = slice(n * NCH, (n + 1) * NCH)
                for f in range(FT):
                    ph1 = php.tile([P, NCH], f32, tag="ph1")
                    ph3 = php.tile([P, NCH], f32, tag="ph3")
                    fcols = slice(f * P, (f + 1) * P)
                    for d in range(DT):
                        nc.tensor.matmul(
                            ph1[:],
                            lhsT=mmv(w1_sb[:, d, fcols]),
                            rhs=mmv(xg_sb[:, d, ncols]),
                            start=(d == 0),
                            stop=(d == DT - 1),
                        )
                    for d in range(DT):
                        nc.tensor.matmul(
                            ph3[:],
                            lhsT=mmv(w3_sb[:, d, fcols]),
                            rhs=mmv(xg_sb[:, d, ncols]),
                            start=(d == 0),
                            stop=(d == DT - 1),
                        )
                    h1 = htmp.tile([P, NCH], f32, tag="h1")
                    nc.vector.tensor_scalar_add(h1[:], ph1[:], b1_sb[:, f : f + 1])
                    prod = htmp.tile([P, NCH], f32, tag="prod")
                    nc.vector.scalar_tensor_tensor(
                        prod[:],
                        in0=ph3[:],
                        scalar=b3_sb[:, f : f + 1],
                        in1=h1[:],
                        op0=mybir.AluOpType.add,
                        op1=mybir.AluOpType.mult,
                    )
                    nc.scalar.activation(
                        a_n[:, f, :], prod[:], mybir.ActivationFunctionType.Silu
                    )
                # second layer for this chunk: yT[:, ncols]
                for d in range(DT):
                    py = pyp.tile([P, NCH], f32, tag="py")
                    dcols = slice(d * P, (d + 1) * P)
                    for f in range(FT):
                        nc.tensor.matmul(
                            py[:],
                            lhsT=mmv(w2_sb[:, f, dcols]),
                            rhs=mmv(a_n[:, f, :]),
                            start=(f == 0),
                            stop=(f == FT - 1),
                        )
                    yo = ytmp.tile([P, NCH], f32, tag="yo")
                    # (psum + b2) * gate_weight
                    nc.vector.scalar_tensor_tensor(
                        yo[:],
                        in0=py[:],
                        scalar=b2_sb[:, d : d + 1],
                        in1=gw_sb[:, ncols],
                        op0=mybir.AluOpType.add,
                        op1=mybir.AluOpType.mult,
                    )
                    nc.sync.dma_start(
                        yT[:].rearrange("(o p) c -> p o c", p=P)[:, d, ncols], yo[:]
                    )

            # ---------- shared expert (512 tokens) ----------
            xs_sb = xgp.tile([P, DT, TS], sdt, tag="xs")
            nc.sync.dma_start(xs_sb[:], xsT[:].rearrange("(o p) c -> p o c", p=P))

            # stream Ws1/Ws3 in column groups of 256 (2 f-tiles per group)
            for g in range(FST // 2):
                gcols = slice(g * 2 * P, (g + 1) * 2 * P)
                ws1_g = wstream.tile([P, DT, 2 * P], sdt, tag="ws1g")
                nc.sync.dma_start(
                    ws1_g[:], ws1[:].rearrange("(o p) f -> p o f", p=P)[:, :, gcols]
                )
                ws3_g = wstream.tile([P, DT, 2 * P], sdt, tag="ws3g")
                nc.sync.dma_start(
                    ws3_g[:], ws3[:].rearrange("(o p) f -> p o f", p=P)[:, :, gcols]
                )
                for sub in range(2):
                    fs = g * 2 + sub
                    scols = slice(sub * P, (sub + 1) * P)
                    ph1 = php.tile([P, TS], f32, tag="ph1")
                    ph3 = php.tile([P, TS], f32, tag="ph3")
                    for d in range(DT):
                        nc.tensor.matmul(
                            ph1[:],
                            lhsT=mmv(ws1_g[:, d, scols]),
                            rhs=mmv(xs_sb[:, d, :]),
                            start=(d == 0),
                            stop=(d == DT - 1),
                        )
                    for d in range(DT):
                        nc.tensor.matmul(
                            ph3[:],
                            lhsT=mmv(ws3_g[:, d, scols]),
                            rhs=mmv(xs_sb[:, d, :]),
                            start=(d == 0),
                            stop=(d == DT - 1),
                        )
                    h1 = htmp.tile([P, TS], f32, tag="h1")
                    nc.vector.tensor_scalar_add(h1[:], ph1[:], bs1_sb[:, fs : fs + 1])
                    prod = htmp.tile([P, TS], f32, tag="prod")
                    nc.vector.scalar_tensor_tensor(
                        prod[:],
                        in0=ph3[:],
                        scalar=bs3_sb[:, fs : fs + 1],
                        in1=h1[:],
                        op0=mybir.AluOpType.add,
                        op1=mybir.AluOpType.mult,
                    )
                    nc.scalar.activation(
                        a_shared[:, fs, :], prod[:], mybir.ActivationFunctionType.Silu
                    )

            # ws2: stream f-row blocks, accumulate ys over all 32 f-tiles
            ws2_sb = wres.tile([P, FST // 2, D], sdt, tag="w1res")  # reuse w1 slot
            nc.sync.dma_start(
                ws2_sb[:],
                ws2[:].rearrange("(o p) d -> p o d", p=P)[:, : FST // 2, :],
            )
            ws2_sb2 = wres.tile([P, FST // 2, D], sdt, tag="w3res")  # reuse w3 slot
            nc.sync.dma_start(
                ws2_sb2[:],
                ws2[:].rearrange("(o p) d -> p o d", p=P)[:, FST // 2 :, :],
            )
            for d in range(DT):
                py = pyp.tile([P, TS], f32, tag="py")
                dcols = slice(d * P, (d + 1) * P)
                for fs in range(FST):
                    src = ws2_sb if fs < FST // 2 else ws2_sb2
                    nc.tensor.matmul(
                        py[:],
                        lhsT=mmv(src[:, fs % (FST // 2), dcols]),
                        rhs=mmv(a_shared[:, fs, :]),
                        start=(fs == 0),
                        stop=(fs == FST - 1),
                    )
                yo = ytmp.tile([P, TS], f32, tag="yo")
                nc.vector.tensor_scalar_add(yo[:], py[:], bs2_sb[:, d : d + 1])
                nc.sync.dma_start(
                    ysT[:].rearrange("(o p) c -> p o c", p=P)[:, d, :], yo[:]
                )

    nc.compile()
    return nc


def _get_program(C, cfg):
    key = (C, cfg)
    if key not in _COMPILED:
        _COMPILED[key] = build_program(C, cfg)
    return _COMPILED[key]


def _route(xf, Wg):
    """Host gating: softmax -> top-2 -> renormalized weights (float64)."""
    logits = xf.astype(np.float64) @ Wg.astype(np.float64)
    m = logits.max(-1, keepdims=True)
    p = np.exp(logits - m)
    scores = p / p.sum(-1, keepdims=True)
    eidx = np.argsort(-scores, axis=-1)[:, :TOPK]
    sel = np.take_along_axis(scores, eidx, -1)
    sm = sel.max(-1, keepdims=True)
    pe = np.exp(sel - sm)
    ew = pe / pe.sum(-1, keepdims=True)
    return eidx, ew.astype(np.float32)


def prepare_in_maps(x, Wg, W1, b1, W3, b3, W2, b2, Ws1, bs1, Ws3, bs3, Ws2, bs2, cfg=MM_CFG):
    xf = np.ascontiguousarray(np.asarray(x, dtype=np.float32).reshape(-1, D))
    eidx, ew = _route(xf, np.asarray(Wg, dtype=np.float32))

    # dispatch lists per expert
    idx_lists = []
    gw_lists = []
    flat_e = eidx.reshape(-1)
    flat_w = ew.reshape(-1)
    tok = np.repeat(np.arange(T), TOPK)
    order = np.argsort(flat_e, kind="stable")
    se, st, sw = flat_e[order], tok[order], flat_w[order]
    counts = np.bincount(se, minlength=E)
    offs = np.concatenate([[0], np.cumsum(counts)])
    for e in range(E):
        idx_lists.append(st[offs[e] : offs[e + 1]])
        gw_lists.append(sw[offs[e] : offs[e + 1]])

    C = max(NCH, int(np.ceil(counts.max() / NCH)) * NCH)

    np_mdt = _np_mm_dtype(cfg)
    in_maps = []
    for e in range(E):
        cnt = counts[e]
        xg = np.zeros((C, D), dtype=np.float32)
        xg[:cnt] = xf[idx_lists[e]]
        gwv = np.zeros((1, C), dtype=np.float32)
        gwv[0, :cnt] = gw_lists[e]
        xsl = xf[e * TS : (e + 1) * TS]
        in_maps.append(
            {
                "xgT": np.ascontiguousarray(xg.T).astype(np_mdt),
                "gw": gwv,
                "w1": np.ascontiguousarray(W1[e]).astype(np_mdt),
                "w3": np.ascontiguousarray(W3[e]).astype(np_mdt),
                "w2": np.ascontiguousarray(W2[e]).astype(np_mdt),
                "b1": np.asarray(b1[e], dtype=np.float32),
                "b3": np.asarray(b3[e], dtype=np.float32),
                "b2": np.asarray(b2[e], dtype=np.float32),
                "xsT": np.ascontiguousarray(xsl.T).astype(np_mdt),
                "ws1": np.asarray(Ws1).astype(np_mdt),
                "ws3": np.asarray(Ws3).astype(np_mdt),
                "ws2": np.asarray(Ws2).astype(np_mdt),
                "bs1": np.asarray(bs1, dtype=np.float32),
                "bs3": np.asarray(bs3, dtype=np.float32),
                "bs2": np.asarray(bs2, dtype=np.float32),
            }
        )
    return in_maps, idx_lists, counts, C


def combine(results, idx_lists, counts, x_shape, x_dtype):
    y = np.empty((D, T), dtype=np.float32)
    for e in range(E):
        y[:, e * TS : (e + 1) * TS] = results[e]["ysT"]
    for e in range(E):
        cnt = counts[e]
        if cnt:
            cols = idx_lists[e][:cnt]
            y[:, cols] += results[e]["yT"][:, :cnt]
    return np.ascontiguousarray(y.T).reshape(x_shape).astype(x_dtype, copy=False)


def run(x, Wg, W1, b1, W3, b3, W2, b2, Ws1, bs1, Ws3, bs3, Ws2, bs2,
        cfg=MM_CFG, trace=False, trace_kwargs=None):
    in_maps, idx_lists, counts, C = prepare_in_maps(
        x, Wg, W1, b1, W3, b3, W2, b2, Ws1, bs1, Ws3, bs3, Ws2, bs2, cfg
    )
    nc = _get_program(C, cfg)
    res = run_bass_kernel_spmd(
        nc, in_maps, list(range(E)), trace=trace, **(trace_kwargs or {})
    )
    out = combine(res.results, idx_lists, counts, np.asarray(x).shape, np.asarray(x).dtype)
    return out, res


def kernel(**inputs):
    out, _ = run(**inputs)
    return out


# revision 7
# speedup vs baseline: 1.0658x; 1.0658x over previous
"""MoE layer (top-2 of 8 experts + shared expert) as a Bass/Tile kernel on 8 TRN2 cores.

Strategy (expert parallelism, per the sharding hint):
  - Host computes the tiny gating network (softmax -> top-2 -> renormalize) and
    builds the all-to-all token dispatch: core e receives the tokens routed to
    expert e (padded to a small fixed capacity C), pre-transposed to [d_model, C].
  - Core e runs expert e's FFN on its tokens (silu(x@W1 * x@W3) @ W2), scales
    each output token by its gate weight, and also runs a 512-token slice of the
    shared expert (token-parallel across the 8 cores).
  - Host scatter-adds the two expert contributions per token and the shared
    output back into the full [T, d] result.

All heavy FLOPs (expert FFNs + shared FFN) run on device; the host only does
O(T*E) gating math and O(T*d) data movement.

Device schedule: the shared expert runs FIRST (its weights stream in small
chunks, so the PE starts within ~2us), while the expert-path weights prefetch
underneath it; the expert path then runs entirely from SBUF-resident weights.
"""

import os
import sys

for _p in ("/opt/trn_rl_repo",):
    if _p not in sys.path and os.path.isdir(_p):
        sys.path.insert(0, _p)

import numpy as np
import ml_dtypes

import concourse.bass as bass
import concourse.mybir as mybir
import concourse.tile as tile
from concourse import bacc
from concourse.bass_utils import run_bass_kernel_spmd

P = 128
D = 1024          # d_model
F = 2048          # d_ff per expert
FS = 4096         # shared expert hidden
E = 8             # experts == cores
TOPK = 2
TS = 512          # shared-expert tokens per core (T / 8)
T = 4096

f32 = mybir.dt.float32
bf16 = mybir.dt.bfloat16

MM_CFG = os.environ.get("MOE_MM_CFG", "bf16")

_COMPILED: dict = {}


def _np_mm_dtype(cfg):
    return ml_dtypes.bfloat16 if cfg == "bf16" else np.float32


def _chunks(C):
    """Split C token columns into matmul-N chunks of <=512."""
    out = []
    s = 0
    while s < C:
        w = min(512, C - s)
        out.append((s, w))
        s += w
    return out


def build_program(C: int, cfg: str):
    """Build the per-core Bass program for expert-token capacity C."""
    assert cfg == "bf16"
    assert C % 64 == 0
    sdt = bf16

    nc = bacc.Bacc("TRN2", target_bir_lowering=False, debug=False, num_devices=E)

    # ---- per-core inputs ----
    xgT = nc.dram_tensor("xgT", [D, C], sdt, kind="ExternalInput")
    gw = nc.dram_tensor("gw", [1, C], f32, kind="ExternalInput")
    w1 = nc.dram_tensor("w1", [D, F], sdt, kind="ExternalInput")
    w3 = nc.dram_tensor("w3", [D, F], sdt, kind="ExternalInput")
    w2 = nc.dram_tensor("w2", [F, D], sdt, kind="ExternalInput")
    b1 = nc.dram_tensor("b1", [F], f32, kind="ExternalInput")
    b3 = nc.dram_tensor("b3", [F], f32, kind="ExternalInput")
    b2 = nc.dram_tensor("b2", [D], f32, kind="ExternalInput")
    xsT = nc.dram_tensor("xsT", [D, TS], sdt, kind="ExternalInput")
    ws1 = nc.dram_tensor("ws1", [D, FS], sdt, kind="ExternalInput")
    ws3 = nc.dram_tensor("ws3", [D, FS], sdt, kind="ExternalInput")
    ws2 = nc.dram_tensor("ws2", [FS, D], sdt, kind="ExternalInput")
    bs1 = nc.dram_tensor("bs1", [FS], f32, kind="ExternalInput")
    bs3 = nc.dram_tensor("bs3", [FS], f32, kind="ExternalInput")
    bs2 = nc.dram_tensor("bs2", [D], f32, kind="ExternalInput")

    # ---- per-core outputs ----
    yT = nc.dram_tensor("yT", [D, C], f32, kind="ExternalOutput")
    ysT = nc.dram_tensor("ysT", [D, TS], f32, kind="ExternalOutput")

    DT = D // P    # 8
    FT = F // P    # 16
    FST = FS // P  # 32
    CH = _chunks(C)

    with tile.TileContext(nc) as tc:
        with (
            tc.tile_pool(name="consts", bufs=1) as consts,
            tc.tile_pool(name="xg", bufs=1) as xgp,
            tc.tile_pool(name="wres", bufs=1) as wres,
            tc.tile_pool(name="wstream", bufs=2) as wstream,
            tc.tile_pool(name="w2stream", bufs=3) as w2stream,
            tc.tile_pool(name="abuf", bufs=1) as abufp,
            tc.tile_pool(name="htmp", bufs=3) as htmp,
            tc.tile_pool(name="ytmp", bufs=3) as ytmp,
            tc.tile_pool(name="ph", bufs=2, space="PSUM") as php,
            tc.tile_pool(name="py", bufs=4, space="PSUM") as pyp,
        ):
            # ---------- tiny constants first ----------
            bs1_sb = consts.tile([P, FST], f32)
            nc.sync.dma_start(bs1_sb[:], bs1[:].rearrange("(o p) -> p o", p=P))
            bs3_sb = consts.tile([P, FST], f32)
            nc.sync.dma_start(bs3_sb[:], bs3[:].rearrange("(o p) -> p o", p=P))
            bs2_sb = consts.tile([P, DT], f32)
            nc.sync.dma_start(bs2_sb[:], bs2[:].rearrange("(o p) -> p o", p=P))
            b1_sb = consts.tile([P, FT], f32)
            nc.sync.dma_start(b1_sb[:], b1[:].rearrange("(o p) -> p o", p=P))
            b3_sb = consts.tile([P, FT], f32)
            nc.sync.dma_start(b3_sb[:], b3[:].rearrange("(o p) -> p o", p=P))
            b2_sb = consts.tile([P, DT], f32)
            nc.sync.dma_start(b2_sb[:], b2[:].rearrange("(o p) -> p o", p=P))

            xs_sb = xgp.tile([P, DT, TS], sdt, tag="xs")
            nc.sync.dma_start(xs_sb[:], xsT[:].rearrange("(o p) c -> p o c", p=P))

            # a-buffer: holds a_shared [P, FST, 512] during the shared stage,
            # then two rotating [P, FT, 512] slabs for the expert chunks.
            ab = abufp.tile([P, FST * 512], sdt, tag="abuf")
            a_shared = ab[:, : FST * 512].rearrange("p (f n) -> p f n", f=FST)

            def a_expert(n):
                off = (n % 2) * (FT * 512)
                return ab[:, off : off + FT * 512].rearrange("p (f n) -> p f n", f=FT)

            # ---------- shared expert: h1s/h3s -> a_shared ----------
            for g in range(FST // 2):
                gcols = slice(g * 2 * P, (g + 1) * 2 * P)
                ws1_g = wstream.tile([P, DT, 2 * P], sdt, tag="ws1g")
                nc.sync.dma_start(
                    ws1_g[:], ws1[:].rearrange("(o p) f -> p o f", p=P)[:, :, gcols]
                )
                ws3_g = wstream.tile([P, DT, 2 * P], sdt, tag="ws3g")
                nc.sync.dma_start(
                    ws3_g[:], ws3[:].rearrange("(o p) f -> p o f", p=P)[:, :, gcols]
                )
                for sub in range(2):
                    fs = g * 2 + sub
                    scols = slice(sub * P, (sub + 1) * P)
                    ph1 = php.tile([P, TS], f32, tag="ph1")
                    ph3 = php.tile([P, TS], f32, tag="ph3")
                    for d in range(DT):
                        nc.tensor.matmul(
                            ph1[:],
                            lhsT=ws1_g[:, d, scols],
                            rhs=xs_sb[:, d, :],
                            start=(d == 0),
                            stop=(d == DT - 1),
                        )
                    for d in range(DT):
                        nc.tensor.matmul(
                            ph3[:],
                            lhsT=ws3_g[:, d, scols],
                            rhs=xs_sb[:, d, :],
                            start=(d == 0),
                            stop=(d == DT - 1),
                        )
                    h1 = htmp.tile([P, TS], f32, tag="h1")
                    nc.vector.tensor_scalar_add(h1[:], ph1[:], bs1_sb[:, fs : fs + 1])
                    prod = htmp.tile([P, TS], f32, tag="prod")
                    nc.vector.scalar_tensor_tensor(
                        prod[:],
                        in0=ph3[:],
                        scalar=bs3_sb[:, fs : fs + 1],
                        in1=h1[:],
                        op0=mybir.AluOpType.add,
                        op1=mybir.AluOpType.mult,
                    )
                    nc.scalar.activation(
                        a_shared[:, fs, :], prod[:], mybir.ActivationFunctionType.Silu
                    )

            # ---------- shared expert: ys = a_shared @ Ws2 ----------
            # d split into two halves of 4; Ws2 streamed per f-row block
            # (re-streamed once per half) so nothing needs residency.
            for dh in range(2):
                pys = [pyp.tile([P, TS], f32, tag="py", name=f"pys_{dh}_{i}") for i in range(4)]
                for fs in range(FST):
                    ws2_b = w2stream.tile([P, D], sdt, tag="ws2b")
                    nc.sync.dma_start(
                        ws2_b[:], ws2[:].rearrange("(o p) d -> p o d", p=P)[:, fs, :]
                    )
                    for i in range(4):
                        d = dh * 4 + i
                        nc.tensor.matmul(
                            pys[i][:],
                            lhsT=ws2_b[:, d * P : (d + 1) * P],
                            rhs=a_shared[:, fs, :],
                            start=(fs == 0),
                            stop=(fs == FST - 1),
                        )
                for i in range(4):
                    d = dh * 4 + i
                    yo = ytmp.tile([P, TS], f32, tag="yo")
                    nc.vector.tensor_scalar_add(yo[:], pys[i][:], bs2_sb[:, d : d + 1])
                    nc.sync.dma_start(
                        ysT[:].rearrange("(o p) c -> p o c", p=P)[:, d, :], yo[:]
                    )

            # ---------- expert-path inputs (prefetch during shared stage) ----------
            xg_sb = xgp.tile([P, DT, C], sdt)
            w1_sb = wres.tile([P, DT, F], sdt, tag="w1res")
            w3_sb = wres.tile([P, DT, F], sdt, tag="w3res")
            w2_sb = wres.tile([P, FT, D], sdt, tag="w2res")
            for d in range(DT):
                nc.sync.dma_start(
                    xg_sb[:, d, :], xgT[:].rearrange("(o p) c -> p o c", p=P)[:, d, :]
                )
                nc.sync.dma_start(
                    w1_sb[:, d, :], w1[:].rearrange("(o p) f -> p o f", p=P)[:, d, :]
                )
                nc.sync.dma_start(
                    w3_sb[:, d, :], w3[:].rearrange("(o p) f -> p o f", p=P)[:, d, :]
                )
            for ft in range(FT):
                nc.sync.dma_start(
                    w2_sb[:, ft, :], w2[:].rearrange("(o p) d -> p o d", p=P)[:, ft, :]
                )
            gw_sb = consts.tile([P, C], f32, tag="gw")
            nc.sync.dma_start(gw_sb[:], gw[:].to_broadcast([P, C]))

            # ---------- expert path ----------
            for n, (cs, cw) in enumerate(CH):
                a_n = a_expert(n)
                ncols = slice(cs, cs + cw)
                for f in range(FT):
                    ph1 = php.tile([P, TS], f32, tag="ph1")
                    ph3 = php.tile([P, TS], f32, tag="ph3")
                    fcols = slice(f * P, (f + 1) * P)
                    for d in range(DT):
                        nc.tensor.matmul(
                            ph1[:, :cw],
                            lhsT=w1_sb[:, d, fcols],
                            rhs=xg_sb[:, d, ncols],
                            start=(d == 0),
                            stop=(d == DT - 1),
                        )
                    for d in range(DT):
                        nc.tensor.matmul(
                            ph3[:, :cw],
                            lhsT=w3_sb[:, d, fcols],
                            rhs=xg_sb[:, d, ncols],
                            start=(d == 0),
                            stop=(d == DT - 1),
                        )
                    h1 = htmp.tile([P, TS], f32, tag="h1")
                    nc.vector.tensor_scalar_add(
                        h1[:, :cw], ph1[:, :cw], b1_sb[:, f : f + 1]
                    )
                    prod = htmp.tile([P, TS], f32, tag="prod")
                    nc.vector.scalar_tensor_tensor(
                        prod[:, :cw],
                        in0=ph3[:, :cw],
                        scalar=b3_sb[:, f : f + 1],
                        in1=h1[:, :cw],
                        op0=mybir.AluOpType.add,
                        op1=mybir.AluOpType.mult,
                    )
                    nc.scalar.activation(
                        a_n[:, f, :cw], prod[:, :cw], mybir.ActivationFunctionType.Silu
                    )
                for d in range(DT):
                    py = pyp.tile([P, TS], f32, tag="py")
                    dcols = slice(d * P, (d + 1) * P)
                    for f in range(FT):
                        nc.tensor.matmul(
                            py[:, :cw],
                            lhsT=w2_sb[:, f, dcols],
                            rhs=a_n[:, f, :cw],
                            start=(f == 0),
                            stop=(f == FT - 1),
                        )
                    yo = ytmp.tile([P, TS], f32, tag="yo")
                    nc.vector.scalar_tensor_tensor(
                        yo[:, :cw],
                        in0=py[:, :cw],
                        scalar=b2_sb[:, d : d + 1],
                        in1=gw_sb[:, ncols],
                        op0=mybir.AluOpType.add,
                        op1=mybir.AluOpType.mult,
                    )
                    nc.sync.dma_start(
                        yT[:].rearrange("(o p) c -> p o c", p=P)[:, d, ncols],
                        yo[:, :cw],
                    )

    nc.compile()
    return nc


def _get_program(C, cfg):
    key = (C, cfg)
    if key not in _COMPILED:
        _COMPILED[key] = build_program(C, cfg)
    return _COMPILED[key]


def _route(xf, Wg):
    """Host gating: softmax -> top-2 -> renormalized weights (float64)."""
    logits = xf.astype(np.float64) @ Wg.astype(np.float64)
    m = logits.max(-1, keepdims=True)
    p = np.exp(logits - m)
    scores = p / p.sum(-1, keepdims=True)
    eidx = np.argsort(-scores, axis=-1, kind="stable")[:, :TOPK]
    sel = np.take_along_axis(scores, eidx, -1)
    sm = sel.max(-1, keepdims=True)
    pe = np.exp(sel - sm)
    ew = pe / pe.sum(-1, keepdims=True)
    return eidx, ew.astype(np.float32)


def prepare_in_maps(x, Wg, W1, b1, W3, b3, W2, b2, Ws1, bs1, Ws3, bs3, Ws2, bs2, cfg=MM_CFG):
    xf = np.ascontiguousarray(np.asarray(x, dtype=np.float32).reshape(-1, D))
    eidx, ew = _route(xf, np.asarray(Wg, dtype=np.float32))

    flat_e = eidx.reshape(-1)
    flat_w = ew.reshape(-1)
    tok = np.repeat(np.arange(T), TOPK)
    order = np.argsort(flat_e, kind="stable")
    se, st, sw = flat_e[order], tok[order], flat_w[order]
    counts = np.bincount(se, minlength=E)
    offs = np.concatenate([[0], np.cumsum(counts)])
    idx_lists = [st[offs[e] : offs[e + 1]] for e in range(E)]
    gw_lists = [sw[offs[e] : offs[e + 1]] for e in range(E)]

    C = max(512, int(np.ceil(counts.max() / 64)) * 64)

    np_mdt = _np_mm_dtype(cfg)
    in_maps = []
    for e in range(E):
        cnt = counts[e]
        xg = np.zeros((C, D), dtype=np.float32)
        xg[:cnt] = xf[idx_lists[e]]
        gwv = np.zeros((1, C), dtype=np.float32)
        gwv[0, :cnt] = gw_lists[e]
        xsl = xf[e * TS : (e + 1) * TS]
        in_maps.append(
            {
                "xgT": np.ascontiguousarray(xg.T).astype(np_mdt),
                "gw": gwv,
                "w1": np.ascontiguousarray(W1[e]).astype(np_mdt),
                "w3": np.ascontiguousarray(W3[e]).astype(np_mdt),
                "w2": np.ascontiguousarray(W2[e]).astype(np_mdt),
                "b1": np.asarray(b1[e], dtype=np.float32),
                "b3": np.asarray(b3[e], dtype=np.float32),
                "b2": np.asarray(b2[e], dtype=np.float32),
                "xsT": np.ascontiguousarray(xsl.T).astype(np_mdt),
                "ws1": np.asarray(Ws1).astype(np_mdt),
                "ws3": np.asarray(Ws3).astype(np_mdt),
                "ws2": np.asarray(Ws2).astype(np_mdt),
                "bs1": np.asarray(bs1, dtype=np.float32),
                "bs3": np.asarray(bs3, dtype=np.float32),
                "bs2": np.asarray(bs2, dtype=np.float32),
            }
        )
    return in_maps, idx_lists, counts, C


def combine(results, idx_lists, counts, x_shape, x_dtype):
    y = np.empty((D, T), dtype=np.float32)
    for e in range(E):
        y[:, e * TS : (e + 1) * TS] = results[e]["ysT"]
    for e in range(E):
        cnt = counts[e]
        if cnt:
            cols = idx_lists[e][:cnt]
            y[:, cols] += results[e]["yT"][:, :cnt]
    return np.ascontiguousarray(y.T).reshape(x_shape).astype(x_dtype, copy=False)


def run(x, Wg, W1, b1, W3, b3, W2, b2, Ws1, bs1, Ws3, bs3, Ws2, bs2,
        cfg=MM_CFG, trace=False, trace_kwargs=None):
    in_maps, idx_lists, counts, C = prepare_in_maps(
        x, Wg, W1, b1, W3, b3, W2, b2, Ws1, bs1, Ws3, bs3, Ws2, bs2, cfg
    )
    nc = _get_program(C, cfg)
    res = run_bass_kernel_spmd(
        nc, in_maps, list(range(E)), trace=trace, **(trace_kwargs or {})
    )
    out = combine(res.results, idx_lists, counts, np.asarray(x).shape, np.asarray(x).dtype)
    return out, res


def kernel(**inputs):
    out, _ = run(**inputs)
    return out


# revision 12
# speedup vs baseline: 1.1432x; 1.0726x over previous
"""MoE layer (top-2 of 8 experts + shared expert) as a Bass/Tile kernel on 8 TRN2 cores.

Strategy (expert parallelism, per the sharding hint):
  - Host computes the tiny gating network (softmax -> top-2 -> renormalize) and
    builds the all-to-all token dispatch: core e receives the tokens routed to
    expert e (padded to a small fixed capacity C), pre-transposed to [d_model, C].
  - Core e runs expert e's FFN on its tokens (silu(x@W1 * x@W3) @ W2), scales
    each output token by its gate weight, and also runs a 512-token slice of the
    shared expert (token-parallel across the 8 cores).
  - Host scatter-adds the two expert contributions per token and the shared
    output back into the full [T, d] result.

All heavy FLOPs (expert FFNs + shared FFN) run on device; the host only does
O(T*E) gating math and O(T*d) data movement.

Device schedule: the shared expert runs FIRST (its weights stream in small
chunks, so the PE starts within ~2us), while the expert-path weights prefetch
underneath it; the expert path then runs entirely from SBUF-resident weights.
"""

import os
import sys

for _p in ("/opt/trn_rl_repo",):
    if _p not in sys.path and os.path.isdir(_p):
        sys.path.insert(0, _p)

import numpy as np
import ml_dtypes

import concourse.bass as bass
import concourse.mybir as mybir
import concourse.tile as tile
from concourse import bacc
from concourse.bass_utils import run_bass_kernel_spmd

P = 128
D = 1024          # d_model
F = 2048          # d_ff per expert
FS = 4096         # shared expert hidden
E = 8             # experts == cores
TOPK = 2
TS = 512          # shared-expert tokens per core (T / 8)
T = 4096

f32 = mybir.dt.float32
bf16 = mybir.dt.bfloat16

MM_CFG = os.environ.get("MOE_MM_CFG", "bf16")

_COMPILED: dict = {}


def _np_mm_dtype(cfg):
    return ml_dtypes.bfloat16 if cfg == "bf16" else np.float32


def _chunks(C):
    """Split C token columns into matmul-N chunks of <=512."""
    out = []
    s = 0
    while s < C:
        w = min(512, C - s)
        out.append((s, w))
        s += w
    return out


def build_program(C: int, cfg: str):
    """Build the per-core Bass program for expert-token capacity C."""
    assert cfg == "bf16"
    assert C % 64 == 0
    sdt = bf16

    nc = bacc.Bacc("TRN2", target_bir_lowering=False, debug=False, num_devices=E)

    # ---- per-core inputs ----
    # biases arrive host-prepacked as [128, K/128] (partition-major) so the
    # DMA is a clean per-partition contiguous read instead of 4-byte gathers
    xgT = nc.dram_tensor("xgT", [D, C], sdt, kind="ExternalInput")
    gw = nc.dram_tensor("gw", [1, C], f32, kind="ExternalInput")
    w1 = nc.dram_tensor("w1", [D, F], sdt, kind="ExternalInput")
    w3 = nc.dram_tensor("w3", [D, F], sdt, kind="ExternalInput")
    w2 = nc.dram_tensor("w2", [F, D], sdt, kind="ExternalInput")
    b1 = nc.dram_tensor("b1", [P, F // P], f32, kind="ExternalInput")
    b3 = nc.dram_tensor("b3", [P, F // P], f32, kind="ExternalInput")
    b2 = nc.dram_tensor("b2", [P, D // P], f32, kind="ExternalInput")
    xsT = nc.dram_tensor("xsT", [D, TS], sdt, kind="ExternalInput")
    ws1 = nc.dram_tensor("ws1", [D, FS], sdt, kind="ExternalInput")
    ws3 = nc.dram_tensor("ws3", [D, FS], sdt, kind="ExternalInput")
    ws2 = nc.dram_tensor("ws2", [FS, D], sdt, kind="ExternalInput")
    bs1 = nc.dram_tensor("bs1", [P, FS // P], f32, kind="ExternalInput")
    bs3 = nc.dram_tensor("bs3", [P, FS // P], f32, kind="ExternalInput")
    bs2 = nc.dram_tensor("bs2", [P, D // P], f32, kind="ExternalInput")

    # ---- per-core outputs ----
    yT = nc.dram_tensor("yT", [D, C], f32, kind="ExternalOutput")
    ysT = nc.dram_tensor("ysT", [D, TS], f32, kind="ExternalOutput")

    DT = D // P    # 8
    FT = F // P    # 16
    FST = FS // P  # 32
    CH = _chunks(C)

    with tile.TileContext(nc) as tc:
        with (
            tc.tile_pool(name="consts", bufs=1) as consts,
            tc.tile_pool(name="xg", bufs=1) as xgp,
            tc.tile_pool(name="wres", bufs=1) as wres,
            tc.tile_pool(name="wstream", bufs=2) as wstream,
            tc.tile_pool(name="w2stream", bufs=3) as w2stream,
            tc.tile_pool(name="abuf", bufs=1) as abufp,
            tc.tile_pool(name="htmp", bufs=3) as htmp,
            tc.tile_pool(name="ytmp", bufs=3) as ytmp,
            tc.tile_pool(name="ph", bufs=2, space="PSUM") as php,
            tc.tile_pool(name="py", bufs=4, space="PSUM") as pyp,
        ):
            # ---------- shared-expert inputs first (PE starts on these) ----------
            xs_sb = xgp.tile([P, DT, TS], sdt, tag="xs")
            nc.sync.dma_start(xs_sb[:], xsT[:].rearrange("(o p) c -> p o c", p=P))
            bs1_sb = consts.tile([P, FST], f32)
            nc.sync.dma_start(bs1_sb[:], bs1[:])
            bs3_sb = consts.tile([P, FST], f32)
            nc.sync.dma_start(bs3_sb[:], bs3[:])
            bs2_sb = consts.tile([P, DT], f32)
            nc.sync.dma_start(bs2_sb[:], bs2[:])

            # a-buffer: holds a_shared [P, FST, 512] during the shared stage,
            # then two rotating [P, FT, 512] slabs for the expert chunks.
            ab = abufp.tile([P, FST * 512], sdt, tag="abuf")
            a_shared = ab[:, : FST * 512].rearrange("p (f n) -> p f n", f=FST)

            def a_expert(n):
                off = (n % 2) * (FT * 512)
                return ab[:, off : off + FT * 512].rearrange("p (f n) -> p f n", f=FT)

            # ---------- shared expert: h1s/h3s -> a_shared ----------
            for g in range(FST // 2):
                gcols = slice(g * 2 * P, (g + 1) * 2 * P)
                ws1_g = wstream.tile([P, DT, 2 * P], sdt, tag="ws1g")
                nc.sync.dma_start(
                    ws1_g[:], ws1[:].rearrange("(o p) f -> p o f", p=P)[:, :, gcols]
                )
                ws3_g = wstream.tile([P, DT, 2 * P], sdt, tag="ws3g")
                nc.sync.dma_start(
                    ws3_g[:], ws3[:].rearrange("(o p) f -> p o f", p=P)[:, :, gcols]
                )
                for sub in range(2):
                    fs = g * 2 + sub
                    scols = slice(sub * P, (sub + 1) * P)
                    ph1 = php.tile([P, TS], f32, tag="ph1")
                    ph3 = php.tile([P, TS], f32, tag="ph3")
                    for d in range(DT):
                        nc.tensor.matmul(
                            ph1[:],
                            lhsT=ws1_g[:, d, scols],
                            rhs=xs_sb[:, d, :],
                            start=(d == 0),
                            stop=(d == DT - 1),
                        )
                    for d in range(DT):
                        nc.tensor.matmul(
                            ph3[:],
                            lhsT=ws3_g[:, d, scols],
                            rhs=xs_sb[:, d, :],
                            start=(d == 0),
                            stop=(d == DT - 1),
                        )
                    h1 = htmp.tile([P, TS], f32, tag="h1")
                    nc.vector.tensor_scalar_add(h1[:], ph1[:], bs1_sb[:, fs : fs + 1])
                    prod = htmp.tile([P, TS], f32, tag="prod")
                    nc.vector.scalar_tensor_tensor(
                        prod[:],
                        in0=ph3[:],
                        scalar=bs3_sb[:, fs : fs + 1],
                        in1=h1[:],
                        op0=mybir.AluOpType.add,
                        op1=mybir.AluOpType.mult,
                    )
                    nc.scalar.activation(
                        a_shared[:, fs, :], prod[:], mybir.ActivationFunctionType.Silu
                    )

            # ---------- shared expert: ys = a_shared @ Ws2 ----------
            # d split into two halves of 4; Ws2 streamed per f-row block
            # (re-streamed once per half) so nothing needs residency.
            for dh in range(2):
                pys = [pyp.tile([P, TS], f32, tag="py", name=f"pys_{dh}_{i}") for i in range(4)]
                for fs in range(FST):
                    ws2_b = w2stream.tile([P, D], sdt, tag="ws2b")
                    nc.sync.dma_start(
                        ws2_b[:], ws2[:].rearrange("(o p) d -> p o d", p=P)[:, fs, :]
                    )
                    for i in range(4):
                        d = dh * 4 + i
                        nc.tensor.matmul(
                            pys[i][:],
                            lhsT=ws2_b[:, d * P : (d + 1) * P],
                            rhs=a_shared[:, fs, :],
                            start=(fs == 0),
                            stop=(fs == FST - 1),
                        )
                for i in range(4):
                    d = dh * 4 + i
                    yo = ytmp.tile([P, TS], f32, tag="yo")
                    nc.vector.tensor_scalar_add(yo[:], pys[i][:], bs2_sb[:, d : d + 1])
                    nc.sync.dma_start(
                        ysT[:].rearrange("(o p) c -> p o c", p=P)[:, d, :], yo[:]
                    )

            # ---------- expert-path inputs (prefetch during shared stage) ----------
            b1_sb = consts.tile([P, FT], f32)
            nc.sync.dma_start(b1_sb[:], b1[:])
            b3_sb = consts.tile([P, FT], f32)
            nc.sync.dma_start(b3_sb[:], b3[:])
            b2_sb = consts.tile([P, DT], f32)
            nc.sync.dma_start(b2_sb[:], b2[:])
            xg_sb = xgp.tile([P, DT, C], sdt)
            w1_sb = wres.tile([P, DT, F], sdt, tag="w1res")
            w3_sb = wres.tile([P, DT, F], sdt, tag="w3res")
            w2_sb = wres.tile([P, FT, D], sdt, tag="w2res")
            for d in range(DT):
                nc.sync.dma_start(
                    xg_sb[:, d, :], xgT[:].rearrange("(o p) c -> p o c", p=P)[:, d, :]
                )
                nc.sync.dma_start(
                    w1_sb[:, d, :], w1[:].rearrange("(o p) f -> p o f", p=P)[:, d, :]
                )
                nc.sync.dma_start(
                    w3_sb[:, d, :], w3[:].rearrange("(o p) f -> p o f", p=P)[:, d, :]
                )
            for ft in range(FT):
                nc.sync.dma_start(
                    w2_sb[:, ft, :], w2[:].rearrange("(o p) d -> p o d", p=P)[:, ft, :]
                )
            gw_sb = consts.tile([P, C], f32, tag="gw")
            nc.sync.dma_start(gw_sb[:], gw[:].to_broadcast([P, C]))

            # ---------- expert path ----------
            for n, (cs, cw) in enumerate(CH):
                a_n = a_expert(n)
                ncols = slice(cs, cs + cw)
                for f in range(FT):
                    ph1 = php.tile([P, TS], f32, tag="ph1")
                    ph3 = php.tile([P, TS], f32, tag="ph3")
                    fcols = slice(f * P, (f + 1) * P)
                    for d in range(DT):
                        nc.tensor.matmul(
                            ph1[:, :cw],
                            lhsT=w1_sb[:, d, fcols],
                            rhs=xg_sb[:, d, ncols],
                            start=(d == 0),
                            stop=(d == DT - 1),
                        )
                    for d in range(DT):
                        nc.tensor.matmul(
                            ph3[:, :cw],
                            lhsT=w3_sb[:, d, fcols],
                            rhs=xg_sb[:, d, ncols],
                            start=(d == 0),
                            stop=(d == DT - 1),
                        )
                    h1 = htmp.tile([P, TS], f32, tag="h1")
                    nc.vector.tensor_scalar_add(
                        h1[:, :cw], ph1[:, :cw], b1_sb[:, f : f + 1]
                    )
                    prod = htmp.tile([P, TS], f32, tag="prod")
                    nc.vector.scalar_tensor_tensor(
                        prod[:, :cw],
                        in0=ph3[:, :cw],
                        scalar=b3_sb[:, f : f + 1],
                        in1=h1[:, :cw],
                        op0=mybir.AluOpType.add,
                        op1=mybir.AluOpType.mult,
                    )
                    nc.scalar.activation(
                        a_n[:, f, :cw], prod[:, :cw], mybir.ActivationFunctionType.Silu
                    )
                for d in range(DT):
                    py = pyp.tile([P, TS], f32, tag="py")
                    dcols = slice(d * P, (d + 1) * P)
                    for f in range(FT):
                        nc.tensor.matmul(
                            py[:, :cw],
                            lhsT=w2_sb[:, f, dcols],
                            rhs=a_n[:, f, :cw],
                            start=(f == 0),
                            stop=(f == FT - 1),
                        )
                    yo = ytmp.tile([P, TS], f32, tag="yo")
                    nc.vector.scalar_tensor_tensor(
                        yo[:, :cw],
                        in0=py[:, :cw],
                        scalar=b2_sb[:, d : d + 1],
                        in1=gw_sb[:, ncols],
                        op0=mybir.AluOpType.add,
                        op1=mybir.AluOpType.mult,
                    )
                    nc.sync.dma_start(
                        yT[:].rearrange("(o p) c -> p o c", p=P)[:, d, ncols],
                        yo[:, :cw],
                    )

    nc.compile()
    return nc


def _get_program(C, cfg):
    key = (C, cfg)
    if key not in _COMPILED:
        _COMPILED[key] = build_program(C, cfg)
    return _COMPILED[key]


def _pack_bias(b):
    """[K] -> [128, K/128] partition-major (element (p, o) = b[o*128+p])."""
    b = np.asarray(b, dtype=np.float32)
    return np.ascontiguousarray(b.reshape(-1, P).T)


def _route(xf, Wg):
    """Host gating: softmax -> top-2 -> renormalized weights (float64)."""
    logits = xf.astype(np.float64) @ Wg.astype(np.float64)
    m = logits.max(-1, keepdims=True)
    p = np.exp(logits - m)
    scores = p / p.sum(-1, keepdims=True)
    eidx = np.argsort(-scores, axis=-1, kind="stable")[:, :TOPK]
    sel = np.take_along_axis(scores, eidx, -1)
    sm = sel.max(-1, keepdims=True)
    pe = np.exp(sel - sm)
    ew = pe / pe.sum(-1, keepdims=True)
    return eidx, ew.astype(np.float32)


def prepare_in_maps(x, Wg, W1, b1, W3, b3, W2, b2, Ws1, bs1, Ws3, bs3, Ws2, bs2, cfg=MM_CFG):
    xf = np.ascontiguousarray(np.asarray(x, dtype=np.float32).reshape(-1, D))
    eidx, ew = _route(xf, np.asarray(Wg, dtype=np.float32))

    flat_e = eidx.reshape(-1)
    flat_w = ew.reshape(-1)
    tok = np.repeat(np.arange(T), TOPK)
    order = np.argsort(flat_e, kind="stable")
    se, st, sw = flat_e[order], tok[order], flat_w[order]
    counts = np.bincount(se, minlength=E)
    offs = np.concatenate([[0], np.cumsum(counts)])
    idx_lists = [st[offs[e] : offs[e + 1]] for e in range(E)]
    gw_lists = [sw[offs[e] : offs[e + 1]] for e in range(E)]

    C = max(512, int(np.ceil(counts.max() / 64)) * 64)

    np_mdt = _np_mm_dtype(cfg)
    in_maps = []
    for e in range(E):
        cnt = counts[e]
        xg = np.zeros((C, D), dtype=np.float32)
        xg[:cnt] = xf[idx_lists[e]]
        gwv = np.zeros((1, C), dtype=np.float32)
        gwv[0, :cnt] = gw_lists[e]
        xsl = xf[e * TS : (e + 1) * TS]
        in_maps.append(
            {
                "xgT": np.ascontiguousarray(xg.T).astype(np_mdt),
                "gw": gwv,
                "w1": np.ascontiguousarray(W1[e]).astype(np_mdt),
                "w3": np.ascontiguousarray(W3[e]).astype(np_mdt),
                "w2": np.ascontiguousarray(W2[e]).astype(np_mdt),
                "b1": _pack_bias(b1[e]),
                "b3": _pack_bias(b3[e]),
                "b2": _pack_bias(b2[e]),
                "xsT": np.ascontiguousarray(xsl.T).astype(np_mdt),
                "ws1": np.asarray(Ws1).astype(np_mdt),
                "ws3": np.asarray(Ws3).astype(np_mdt),
                "ws2": np.asarray(Ws2).astype(np_mdt),
                "bs1": _pack_bias(bs1),
                "bs3": _pack_bias(bs3),
                "bs2": _pack_bias(bs2),
            }
        )
    return in_maps, idx_lists, counts, C


def combine(results, idx_lists, counts, x_shape, x_dtype):
    y = np.empty((D, T), dtype=np.float32)
    for e in range(E):
        y[:, e * TS : (e + 1) * TS] = results[e]["ysT"]
    for e in range(E):
        cnt = counts[e]
        if cnt:
            cols = idx_lists[e][:cnt]
            y[:, cols] += results[e]["yT"][:, :cnt]
    return np.ascontiguousarray(y.T).reshape(x_shape).astype(x_dtype, copy=False)


def run(x, Wg, W1, b1, W3, b3, W2, b2, Ws1, bs1, Ws3, bs3, Ws2, bs2,
        cfg=MM_CFG, trace=False, trace_kwargs=None):
    in_maps, idx_lists, counts, C = prepare_in_maps(
        x, Wg, W1, b1, W3, b3, W2, b2, Ws1, bs1, Ws3, bs3, Ws2, bs2, cfg
    )
    nc = _get_program(C, cfg)
    res = run_bass_kernel_spmd(
        nc, in_maps, list(range(E)), trace=trace, **(trace_kwargs or {})
    )
    out = combine(res.results, idx_lists, counts, np.asarray(x).shape, np.asarray(x).dtype)
    return out, res


def kernel(**inputs):
    out, _ = run(**inputs)
    return out


# revision 20
# speedup vs baseline: 1.2234x; 1.0702x over previous
"""MoE layer (top-2 of 8 experts + shared expert) as a Bass/Tile kernel on 8 TRN2 cores.

Strategy (expert parallelism, per the sharding hint):
  - Host computes the tiny gating network (softmax -> top-2 -> renormalize) and
    builds the all-to-all token dispatch: core e receives the tokens routed to
    expert e (padded to a small fixed capacity C), pre-transposed to [d_model, C].
  - Core e runs expert e's FFN on its tokens (silu(x@W1 * x@W3) @ W2), scales
    each output token by its gate weight, and also runs a 512-token slice of the
    shared expert (token-parallel across the 8 cores).
  - Host scatter-adds the two expert contributions per token and the shared
    output back into the full [T, d] result.

All heavy FLOPs (expert FFNs + shared FFN) run on device; the host only does
O(T*E) gating math and O(T*d) data movement.

Device schedule: the shared expert runs FIRST (its weights stream in small
chunks, so the PE starts within ~2us), while the expert-path weights prefetch
underneath it; the expert path then runs entirely from SBUF-resident weights.
"""

import os
import sys

for _p in ("/opt/trn_rl_repo",):
    if _p not in sys.path and os.path.isdir(_p):
        sys.path.insert(0, _p)

import numpy as np
import ml_dtypes

import concourse.bass as bass
import concourse.mybir as mybir
import concourse.tile as tile
from concourse import bacc
from concourse.bass_utils import run_bass_kernel_spmd

P = 128
D = 1024          # d_model
F = 2048          # d_ff per expert
FS = 4096         # shared expert hidden
E = 8             # experts == cores
TOPK = 2
TS = 512          # shared-expert tokens per core (T / 8)
T = 4096

f32 = mybir.dt.float32
bf16 = mybir.dt.bfloat16

MM_CFG = os.environ.get("MOE_MM_CFG", "bf16")

_COMPILED: dict = {}


def _np_mm_dtype(cfg):
    return ml_dtypes.bfloat16 if cfg == "bf16" else np.float32


def _chunks(C):
    """Split C token columns into matmul-N chunks of <=512."""
    out = []
    s = 0
    while s < C:
        w = min(512, C - s)
        out.append((s, w))
        s += w
    return out


def build_program(C: int, cfg: str):
    """Build the per-core Bass program for expert-token capacity C."""
    assert cfg == "bf16"
    assert C % 64 == 0
    sdt = bf16

    nc = bacc.Bacc("TRN2", target_bir_lowering=False, debug=False, num_devices=E)

    # ---- per-core inputs ----
    # All inputs arrive host-pretiled in the exact SBUF layout (partition
    # dim first) so every DMA is one contiguous per-partition segment.
    DT = D // P    # 8
    FT = F // P    # 16
    FST = FS // P  # 32
    WG = 2 * P     # ws1/ws3 streaming group width

    xgT = nc.dram_tensor("xgT", [P, DT, C], sdt, kind="ExternalInput")
    gw = nc.dram_tensor("gw", [1, C], f32, kind="ExternalInput")
    w1 = nc.dram_tensor("w1", [P, DT, F], sdt, kind="ExternalInput")
    w3 = nc.dram_tensor("w3", [P, DT, F], sdt, kind="ExternalInput")
    w2 = nc.dram_tensor("w2", [P, FT, D], sdt, kind="ExternalInput")
    b1 = nc.dram_tensor("b1", [P, F // P], f32, kind="ExternalInput")
    b3 = nc.dram_tensor("b3", [P, F // P], f32, kind="ExternalInput")
    b2 = nc.dram_tensor("b2", [P, D // P], f32, kind="ExternalInput")
    xsT = nc.dram_tensor("xsT", [P, DT, TS], sdt, kind="ExternalInput")
    ws1 = nc.dram_tensor("ws1", [FS // WG, P, DT, WG], sdt, kind="ExternalInput")
    ws3 = nc.dram_tensor("ws3", [FS // WG, P, DT, WG], sdt, kind="ExternalInput")
    ws2 = nc.dram_tensor("ws2", [FS, D], sdt, kind="ExternalInput")
    bs1 = nc.dram_tensor("bs1", [P, FS // P], f32, kind="ExternalInput")
    bs3 = nc.dram_tensor("bs3", [P, FS // P], f32, kind="ExternalInput")
    bs2 = nc.dram_tensor("bs2", [P, D // P], f32, kind="ExternalInput")

    # ---- per-core outputs ----
    yT = nc.dram_tensor("yT", [D, C], f32, kind="ExternalOutput")
    ysT = nc.dram_tensor("ysT", [D, TS], f32, kind="ExternalOutput")

    CH = _chunks(C)

    with tile.TileContext(nc) as tc:
        with (
            tc.tile_pool(name="consts", bufs=1) as consts,
            tc.tile_pool(name="xg", bufs=1) as xgp,
            tc.tile_pool(name="wres", bufs=1) as wres,
            tc.tile_pool(name="wstream", bufs=2) as wstream,
            tc.tile_pool(name="w2stream", bufs=3) as w2stream,
            tc.tile_pool(name="abuf", bufs=1) as abufp,
            tc.tile_pool(name="htmp", bufs=3) as htmp,
            tc.tile_pool(name="ytmp", bufs=3) as ytmp,
            tc.tile_pool(name="ps", bufs=4, space="PSUM") as psp,
        ):
            # ---------- shared-expert inputs first (PE starts on these) ----------
            xs_sb = xgp.tile([P, DT, TS], sdt, tag="xs")
            nc.sync.dma_start(xs_sb[:], xsT[:])
            bs1_sb = consts.tile([P, FST], f32)
            nc.sync.dma_start(bs1_sb[:], bs1[:])
            bs3_sb = consts.tile([P, FST], f32)
            nc.sync.dma_start(bs3_sb[:], bs3[:])
            bs2_sb = consts.tile([P, DT], f32)
            nc.sync.dma_start(bs2_sb[:], bs2[:])

            # a-buffer: holds a_shared [P, FST, 512] during the shared stage,
            # then two rotating [P, FT, 512] slabs for the expert chunks.
            ab = abufp.tile([P, FST * 512], sdt, tag="abuf")
            a_shared = ab[:, : FST * 512].rearrange("p (f n) -> p f n", f=FST)

            def a_expert(n):
                off = (n % 2) * (FT * 512)
                return ab[:, off : off + FT * 512].rearrange("p (f n) -> p f n", f=FT)

            # ---------- shared expert: h1s/h3s -> a_shared ----------
            for g in range(FST // 2):
                ws1_g = wstream.tile([P, DT, WG], sdt, tag="ws1g")
                nc.sync.dma_start(ws1_g[:], ws1[g])
                ws3_g = wstream.tile([P, DT, WG], sdt, tag="ws3g")
                nc.sync.dma_start(ws3_g[:], ws3[g])
                for sub in range(2):
                    fs = g * 2 + sub
                    scols = slice(sub * P, (sub + 1) * P)
                    ph1 = psp.tile([P, TS], f32, tag="ph1")
                    ph3 = psp.tile([P, TS], f32, tag="ph3")
                    for d in range(DT):
                        nc.tensor.matmul(
                            ph1[:],
                            lhsT=ws1_g[:, d, scols],
                            rhs=xs_sb[:, d, :],
                            start=(d == 0),
                            stop=(d == DT - 1),
                        )
                    for d in range(DT):
                        nc.tensor.matmul(
                            ph3[:],
                            lhsT=ws3_g[:, d, scols],
                            rhs=xs_sb[:, d, :],
                            start=(d == 0),
                            stop=(d == DT - 1),
                        )
                    h1 = htmp.tile([P, TS], f32, tag="h1")
                    nc.vector.tensor_scalar_add(h1[:], ph1[:], bs1_sb[:, fs : fs + 1])
                    prod = htmp.tile([P, TS], f32, tag="prod")
                    nc.vector.scalar_tensor_tensor(
                        prod[:],
                        in0=ph3[:],
                        scalar=bs3_sb[:, fs : fs + 1],
                        in1=h1[:],
                        op0=mybir.AluOpType.add,
                        op1=mybir.AluOpType.mult,
                    )
                    nc.scalar.activation(
                        a_shared[:, fs, :], prod[:], mybir.ActivationFunctionType.Silu
                    )

            # ---------- shared expert: ys = a_shared @ Ws2 ----------
            # single pass over Ws2 (streamed once) accumulating all 8 d-tiles
            # in 8 PSUM banks at once
            pys = [
                psp.tile([P, TS], f32, tag=("ph1" if i < 4 else "ph3"), name=f"pys_{i}")
                for i in range(DT)
            ]
            for fs in range(FST):
                ws2_b = w2stream.tile([P, D], sdt, tag="ws2b")
                nc.sync.dma_start(
                    ws2_b[:], ws2[:].rearrange("(o p) d -> p o d", p=P)[:, fs, :]
                )
                for d in range(DT):
                    nc.tensor.matmul(
                        pys[d][:],
                        lhsT=ws2_b[:, d * P : (d + 1) * P],
                        rhs=a_shared[:, fs, :],
                        start=(fs == 0),
                        stop=(fs == FST - 1),
                    )
            for d in range(DT):
                yo = ytmp.tile([P, TS], f32, tag="yo")
                nc.vector.tensor_scalar_add(yo[:], pys[d][:], bs2_sb[:, d : d + 1])
                nc.sync.dma_start(
                    ysT[:].rearrange("(o p) c -> p o c", p=P)[:, d, :], yo[:]
                )

            # ---------- expert-path inputs (prefetch during shared stage) ----------
            b1_sb = consts.tile([P, FT], f32)
            nc.sync.dma_start(b1_sb[:], b1[:])
            b3_sb = consts.tile([P, FT], f32)
            nc.sync.dma_start(b3_sb[:], b3[:])
            b2_sb = consts.tile([P, DT], f32)
            nc.sync.dma_start(b2_sb[:], b2[:])
            xg_sb = xgp.tile([P, DT, C], sdt)
            nc.sync.dma_start(xg_sb[:], xgT[:])
            w1_sb = wres.tile([P, DT, F], sdt, tag="w1res")
            nc.sync.dma_start(w1_sb[:], w1[:])
            w3_sb = wres.tile([P, DT, F], sdt, tag="w3res")
            nc.sync.dma_start(w3_sb[:], w3[:])
            w2_sb = wres.tile([P, FT, D], sdt, tag="w2res")
            nc.sync.dma_start(w2_sb[:], w2[:])
            gw_sb = consts.tile([P, C], f32, tag="gw")
            nc.sync.dma_start(gw_sb[:], gw[:].to_broadcast([P, C]))

            # ---------- expert path ----------
            for n, (cs, cw) in enumerate(CH):
                a_n = a_expert(n)
                ncols = slice(cs, cs + cw)
                for f in range(FT):
                    ph1 = psp.tile([P, TS], f32, tag="ph1")
                    ph3 = psp.tile([P, TS], f32, tag="ph3")
                    fcols = slice(f * P, (f + 1) * P)
                    for d in range(DT):
                        nc.tensor.matmul(
                            ph1[:, :cw],
                            lhsT=w1_sb[:, d, fcols],
                            rhs=xg_sb[:, d, ncols],
                            start=(d == 0),
                            stop=(d == DT - 1),
                        )
                    for d in range(DT):
                        nc.tensor.matmul(
                            ph3[:, :cw],
                            lhsT=w3_sb[:, d, fcols],
                            rhs=xg_sb[:, d, ncols],
                            start=(d == 0),
                            stop=(d == DT - 1),
                        )
                    h1 = htmp.tile([P, TS], f32, tag="h1")
                    nc.vector.tensor_scalar_add(
                        h1[:, :cw], ph1[:, :cw], b1_sb[:, f : f + 1]
                    )
                    prod = htmp.tile([P, TS], f32, tag="prod")
                    nc.vector.scalar_tensor_tensor(
                        prod[:, :cw],
                        in0=ph3[:, :cw],
                        scalar=b3_sb[:, f : f + 1],
                        in1=h1[:, :cw],
                        op0=mybir.AluOpType.add,
                        op1=mybir.AluOpType.mult,
                    )
                    nc.scalar.activation(
                        a_n[:, f, :cw], prod[:, :cw], mybir.ActivationFunctionType.Silu
                    )
                for d in range(DT):
                    py = psp.tile([P, TS], f32, tag="ph1")
                    dcols = slice(d * P, (d + 1) * P)
                    for f in range(FT):
                        nc.tensor.matmul(
                            py[:, :cw],
                            lhsT=w2_sb[:, f, dcols],
                            rhs=a_n[:, f, :cw],
                            start=(f == 0),
                            stop=(f == FT - 1),
                        )
                    yo = ytmp.tile([P, TS], f32, tag="yo")
                    nc.vector.scalar_tensor_tensor(
                        yo[:, :cw],
                        in0=py[:, :cw],
                        scalar=b2_sb[:, d : d + 1],
                        in1=gw_sb[:, ncols],
                        op0=mybir.AluOpType.add,
                        op1=mybir.AluOpType.mult,
                    )
                    nc.sync.dma_start(
                        yT[:].rearrange("(o p) c -> p o c", p=P)[:, d, ncols],
                        yo[:, :cw],
                    )

    nc.compile()
    return nc


def _get_program(C, cfg):
    key = (C, cfg)
    if key not in _COMPILED:
        _COMPILED[key] = build_program(C, cfg)
    return _COMPILED[key]


def _pack_bias(b):
    """[K] -> [128, K/128] partition-major (element (p, o) = b[o*128+p])."""
    b = np.asarray(b, dtype=np.float32)
    return np.ascontiguousarray(b.reshape(-1, P).T)


def _route(xf, Wg):
    """Host gating: softmax -> top-2 -> renormalized weights (float64)."""
    logits = xf.astype(np.float64) @ Wg.astype(np.float64)
    m = logits.max(-1, keepdims=True)
    p = np.exp(logits - m)
    scores = p / p.sum(-1, keepdims=True)
    eidx = np.argsort(-scores, axis=-1, kind="stable")[:, :TOPK]
    sel = np.take_along_axis(scores, eidx, -1)
    sm = sel.max(-1, keepdims=True)
    pe = np.exp(sel - sm)
    ew = pe / pe.sum(-1, keepdims=True)
    return eidx, ew.astype(np.float32)


def prepare_in_maps(x, Wg, W1, b1, W3, b3, W2, b2, Ws1, bs1, Ws3, bs3, Ws2, bs2, cfg=MM_CFG):
    xf = np.ascontiguousarray(np.asarray(x, dtype=np.float32).reshape(-1, D))
    eidx, ew = _route(xf, np.asarray(Wg, dtype=np.float32))

    flat_e = eidx.reshape(-1)
    flat_w = ew.reshape(-1)
    tok = np.repeat(np.arange(T), TOPK)
    order = np.argsort(flat_e, kind="stable")
    se, st, sw = flat_e[order], tok[order], flat_w[order]
    counts = np.bincount(se, minlength=E)
    offs = np.concatenate([[0], np.cumsum(counts)])
    idx_lists = [st[offs[e] : offs[e + 1]] for e in range(E)]
    gw_lists = [sw[offs[e] : offs[e + 1]] for e in range(E)]

    C = max(512, int(np.ceil(counts.max() / 64)) * 64)

    np_mdt = _np_mm_dtype(cfg)
    WG = 2 * P

    def tile_kxn(a, K):
        # [K, N] -> [P, K/P, N] partition-major
        a = np.asarray(a, dtype=np.float32)
        return a.reshape(K // P, P, -1).transpose(1, 0, 2).astype(np_mdt)

    # shared-expert weights are identical on every core: pack once
    ws1_t = np.asarray(Ws1, dtype=np.float32).reshape(D // P, P, FS // WG, WG)
    ws1_t = ws1_t.transpose(2, 1, 0, 3).astype(np_mdt)
    ws3_t = np.asarray(Ws3, dtype=np.float32).reshape(D // P, P, FS // WG, WG)
    ws3_t = ws3_t.transpose(2, 1, 0, 3).astype(np_mdt)
    ws2_t = np.asarray(Ws2, dtype=np.float32).astype(np_mdt)
    bs1_p, bs3_p, bs2_p = _pack_bias(bs1), _pack_bias(bs3), _pack_bias(bs2)

    in_maps = []
    for e in range(E):
        cnt = counts[e]
        xg = np.zeros((C, D), dtype=np.float32)
        xg[:cnt] = xf[idx_lists[e]]
        gwv = np.zeros((1, C), dtype=np.float32)
        gwv[0, :cnt] = gw_lists[e]
        xsl = xf[e * TS : (e + 1) * TS]
        in_maps.append(
            {
                "xgT": tile_kxn(xg.T, D),
                "gw": gwv,
                "w1": tile_kxn(W1[e], D),
                "w3": tile_kxn(W3[e], D),
                "w2": tile_kxn(W2[e], F),
                "b1": _pack_bias(b1[e]),
                "b3": _pack_bias(b3[e]),
                "b2": _pack_bias(b2[e]),
                "xsT": tile_kxn(xsl.T, D),
                "ws1": ws1_t,
                "ws3": ws3_t,
                "ws2": ws2_t,
                "bs1": bs1_p,
                "bs3": bs3_p,
                "bs2": bs2_p,
            }
        )
    return in_maps, idx_lists, counts, C


def combine(results, idx_lists, counts, x_shape, x_dtype):
    y = np.empty((D, T), dtype=np.float32)
    for e in range(E):
        y[:, e * TS : (e + 1) * TS] = results[e]["ysT"]
    for e in range(E):
        cnt = counts[e]
        if cnt:
            cols = idx_lists[e][:cnt]
            y[:, cols] += results[e]["yT"][:, :cnt]
    return np.ascontiguousarray(y.T).reshape(x_shape).astype(x_dtype, copy=False)


def run(x, Wg, W1, b1, W3, b3, W2, b2, Ws1, bs1, Ws3, bs3, Ws2, bs2,
        cfg=MM_CFG, trace=False, trace_kwargs=None):
    in_maps, idx_lists, counts, C = prepare_in_maps(
        x, Wg, W1, b1, W3, b3, W2, b2, Ws1, bs1, Ws3, bs3, Ws2, bs2, cfg
    )
    nc = _get_program(C, cfg)
    res = run_bass_kernel_spmd(
        nc, in_maps, list(range(E)), trace=trace, **(trace_kwargs or {})
    )
    out = combine(res.results, idx_lists, counts, np.asarray(x).shape, np.asarray(x).dtype)
    return out, res


def kernel(**inputs):
    out, _ = run(**inputs)
    return out
